# revision 1
# baseline (speedup 1.0000x reference)
#!/usr/bin/env python3
"""EnvAwareRouter Trainium2 kernel.

Reference computation (per example b):
  t[c]   = gelu(contextual[b,c,:] @ tw1 + tb1) @ tw2 + tb2          (C=13, T=24, H=64)
  logits = gelu(t @ cw1 + cb1) @ cw2 + cb2                          (E=8)
  probs  = softmax(logits - log(-log(u) + eps))
  mask   = k-hot(top-3 probs);  mask_ste = mask + probs - probs

Device strategy (8 cores, pure data parallel over B=524288):
  - host: transpose contextual to [C*T, B] and split fp16 hi/lo
  - h1 = x @ tw1 via fp16 3-term matmuls, tile_position-packed (K=24, M=64)
  - fold tw2 into cw1:  W2[(c,h), j] = tw2[h]*cw1[c,j]  -> one fused
    [832 -> 64] accumulated matmul consuming gelu(h1) (fp16 hi/lo)
  - logits via cw2 hi/lo; gumbel noise term computed on host (log)
  - per-8192 block: PE-transpose logits to natural layout, softmax + rank-count
    top-3 on DVE, gelu/exp ACT table sets switched once per block
"""
import sys

sys.path.insert(0, "/opt/trn_rl_repo")

import numpy as np

import concourse.bass as bass
import concourse.tile as tile
from concourse import bacc, mybir
from concourse.bass_utils import run_bass_kernel_spmd
from contextlib import ExitStack

F32 = mybir.dt.float32
F16 = mybir.dt.float16
AF = mybir.ActivationFunctionType
OP = mybir.AluOpType
AX = mybir.AxisListType

B, C, T, H, E, TOPK = 524288, 13, 24, 64, 8, 3
EPS = 1e-10
N_CORES = 8
BC = B // N_CORES          # 65536 examples per core
BLK = 8192                 # examples per ACT-table block
CHUNK = 512                # examples per compute chunk
DCOLS = 2048               # x DMA granularity (examples)
SC2 = 64.0                 # W2 fp16 scaling
SC3 = 8.0                  # cw2 fp16 scaling
STAGE = 99                 # debug: truncate pipeline after this stage
TRACE = False              # profile core 0 and record LAST_EXEC_NS
LAST_EXEC_NS = None

# --- static c-layout tables ---
RG = [c % 4 for c in range(C)]                    # PE row-group of c
HALF1 = {4, 12, 5, 2, 10, 7}                      # c's whose h1 lands on partitions 64..127
HALF = [1 if c in HALF1 else 0 for c in range(C)]
PAIRS = [(0, 4), (8, 12), (1, 5), (9, 2), (6, 10), (3, 7)]   # (half0, half1) per bank
LONER = 11                                        # bank-6 [0:64]
# h1 psum location of c: (partition offset, free offset in h1big)
H1LOC = {}
for b, (clo, chi) in enumerate(PAIRS):
    H1LOC[clo] = (0, 512 * b)
    H1LOC[chi] = (64, 512 * b)
H1LOC[LONER] = (0, 512 * 6)
# consume chunks: rows of W2 per K=128 chunk
W2CHUNKS = [(clo, chi) for (clo, chi) in PAIRS] + [(LONER, None)]


def _build_program(n_examples=BC):
    """Build the SPMD single-core program (all cores run it identically)."""
    assert n_examples % BLK == 0
    nblk = n_examples // BLK
    W = BLK // 128              # examples per partition per block
    TW = W * E                  # tail tile width
    CW = CHUNK // W             # zT r-columns per chunk
    nc = bacc.Bacc()

    xh_d = nc.declare_dram_parameter("xh", [C * T, n_examples], F16, isOutput=False)
    xl_d = nc.declare_dram_parameter("xl", [C * T, n_examples], F16, isOutput=False)
    gn_d = nc.declare_dram_parameter("gn8", [n_examples, E], F32, isOutput=False)
    w1h_d = nc.declare_dram_parameter("w1h", [128, H], F16, isOutput=False)
    w1l_d = nc.declare_dram_parameter("w1l", [128, H], F16, isOutput=False)
    w2h_d = nc.declare_dram_parameter("w2h", [128, 7 * H], F16, isOutput=False)
    w2l_d = nc.declare_dram_parameter("w2l", [128, 7 * H], F16, isOutput=False)
    cw2_d = nc.declare_dram_parameter("cw2hl", [128, 2 * E], F16, isOutput=False)
    tb1_d = nc.declare_dram_parameter("tb1r", [128, 1], F32, isOutput=False)
    b1p_d = nc.declare_dram_parameter("b1p", [128, 1], F32, isOutput=False)
    ltm_d = nc.declare_dram_parameter("ltm", [128, E * E], F32, isOutput=False)
    ide_d = nc.declare_dram_parameter("ide", [E, E], F32, isOutput=False)
    mask_d = nc.declare_dram_parameter("mask", [n_examples, E], F32, isOutput=True)
    probs_d = nc.declare_dram_parameter("probs", [n_examples, E], F32, isOutput=True)

    with tile.TileContext(nc) as tc, ExitStack() as ctx:
        cpool = ctx.enter_context(tc.tile_pool(name="consts", bufs=1))
        xpool = ctx.enter_context(tc.tile_pool(name="x", bufs=2))
        gpool = ctx.enter_context(tc.tile_pool(name="g1", bufs=2))
        wpool = ctx.enter_context(tc.tile_pool(name="work", bufs=2))
        zpool = ctx.enter_context(tc.tile_pool(name="ztail", bufs=1))
        pspool = ctx.enter_context(tc.tile_pool(name="ps", bufs=1, space="PSUM"))

        # ---- constants ----
        w1h = cpool.tile([128, H], F16, tag="w1h")
        nc.sync.dma_start(out=w1h[:], in_=w1h_d[:])
        w1l = cpool.tile([128, H], F16, tag="w1l")
        nc.sync.dma_start(out=w1l[:], in_=w1l_d[:])
        w2h = cpool.tile([128, 7 * H], F16, tag="w2h")
        nc.sync.dma_start(out=w2h[:], in_=w2h_d[:])
        w2l = cpool.tile([128, 7 * H], F16, tag="w2l")
        nc.sync.dma_start(out=w2l[:], in_=w2l_d[:])
        cw2 = cpool.tile([128, 2 * E], F16, tag="cw2")
        nc.sync.dma_start(out=cw2[:], in_=cw2_d[:])
        tb1 = cpool.tile([128, 1], F32, tag="tb1")
        nc.sync.dma_start(out=tb1[:], in_=tb1_d[:])
        b1p = cpool.tile([128, 1], F32, tag="b1p")
        nc.sync.dma_start(out=b1p[:], in_=b1p_d[:])
        ltm = cpool.tile([128, E * E], F32, tag="ltm")
        nc.sync.dma_start(out=ltm[:], in_=ltm_d[:])
        ide = cpool.tile([E, E], F32, tag="ide")
        nc.sync.dma_start(out=ide[:], in_=ide_d[:])

        # ---- persistent PSUM ----
        h1big = pspool.tile([128, 3584], F32, tag="h1big")   # banks 0-6
        pre2 = pspool.tile([128, 512], F32, tag="pre2")      # bank 7 (consume + tail znat)
        # bank-6 [64:128] is read by the wide gelu before logits ever write it
        nc.vector.memset(h1big[64:128, 3072:3584], 0.0)

        for blk in range(nblk):
            b0 = blk * BLK
            zT = zpool.tile([E, BLK], F32, tag="zT")
            for d in range(BLK // DCOLS):
                col0 = b0 + d * DCOLS
                xh_t = xpool.tile([128, 3, DCOLS], F16, tag="xh")
                xl_t = xpool.tile([128, 3, DCOLS], F16, tag="xl")
                import os as _os
                _ngg = int(_os.environ.get("XGG", "4")) if STAGE >= 0 else 0
                for gg in range(_ngg):
                    src = (xh_d[0 : 12 * T, col0 : col0 + DCOLS]
                           .rearrange("(t c q) n -> t c q n", t=3, c=4)[:, gg]
                           .transpose([1, 0, 2]))
                    nc.sync.dma_start(out=xh_t[32 * gg : 32 * gg + T, :, :], in_=src)
                    srcl = (xl_d[0 : 12 * T, col0 : col0 + DCOLS]
                            .rearrange("(t c q) n -> t c q n", t=3, c=4)[:, gg]
                            .transpose([1, 0, 2]))
                    nc.sync.dma_start(out=xl_t[32 * gg : 32 * gg + T, :, :], in_=srcl)
                xh12 = xpool.tile([32, DCOLS], F16, tag="xh12")
                xl12 = xpool.tile([32, DCOLS], F16, tag="xl12")
                if STAGE >= 0 and int(_os.environ.get("X12", "1")):
                    nc.sync.dma_start(
                        out=xh12[0:T, :], in_=xh_d[12 * T : 13 * T, col0 : col0 + DCOLS]
                    )
                    nc.sync.dma_start(
                        out=xl12[0:T, :], in_=xl_d[12 * T : 13 * T, col0 : col0 + DCOLS]
                    )

                for k in range(DCOLS // CHUNK):
                    kg = d * (DCOLS // CHUNK) + k      # chunk idx in block (0..15)
                    off = k * CHUNK

                    # ---- h1: 3-term fp16 matmuls, term-major so different
                    # tile positions issue back-to-back and run concurrently ----
                    if STAGE < 1:
                        continue
                    for term in range(3):
                        for c in range(C):
                            g, tdx = c % 4, c // 4
                            if c < 12:
                                rh = xh_t[32 * g : 32 * g + T, tdx, off : off + CHUNK]
                                rl = xl_t[32 * g : 32 * g + T, tdx, off : off + CHUNK]
                            else:
                                rh = xh12[0:T, off : off + CHUNK]
                                rl = xl12[0:T, off : off + CHUNK]
                            po, fo = H1LOC[c]
                            out = h1big[po : po + H, fo : fo + CHUNK]
                            tp = (32 * g, 64 * HALF[c])
                            lh = w1h[32 * g : 32 * g + T, :]
                            ll = w1l[32 * g : 32 * g + T, :]
                            w, r = ((lh, rh), (lh, rl), (ll, rh))[term]
                            nc.tensor.matmul(out, w, r, start=(term == 0),
                                             stop=(term == 2), tile_position=tp)

                    # ---- gelu(h1 + tb1) -> g1 fp16 ----
                    if STAGE < 2:
                        continue
                    g1 = gpool.tile([128, 3584], F16, tag="g1")
                    nc.scalar.activation(g1[:, 0:1536], h1big[:, 0:1536], AF.Gelu,
                                         bias=tb1[:])
                    nc.scalar.activation(g1[:, 1536:3584], h1big[:, 1536:3584],
                                         AF.Gelu, bias=tb1[:])

                    if STAGE < 3:
                        continue
                    # ---- consume: pre2 = g1 @ W2; example-half A -> partitions
                    # 0:64 (col grps 0-1), half B -> 64:128 (col grps 2-3) ----
                    for step, (term, j) in enumerate(
                        [(t, j) for t in range(2) for j in range(7)]
                    ):
                        for half in range(2):
                            o2 = half * 256
                            prow = 64 * half
                            wS = (w2h, w2l)[term]
                            kk = 128 if j < 6 else 64
                            lhsT = wS[0:kk, H * j : H * (j + 1)]
                            rhs = g1[0:kk, 512 * j + o2 : 512 * j + o2 + 256]
                            nc.tensor.matmul(
                                pre2[prow : prow + H, 0:256],
                                lhsT, rhs,
                                start=(step == 0), stop=(step == 13),
                                tile_position=(0, prow),
                            )

                    if STAGE < 4:
                        continue
                    # ---- h2 = gelu(pre2/SC2 + b1p), both halves in one op ----
                    h2 = wpool.tile([128, 256], F16, tag="h2")
                    nc.scalar.activation(h2[:], pre2[:, 0:256], AF.Gelu, bias=b1p[:],
                                         scale=1.0 / SC2)

                    # ---- logits8 = SC3 * h2 @ cw2 (into h1big bank6 [64:72]) ----
                    # two concurrent M=8 groups must land in different banks
                    # (same-partition same-bank concurrent drains wedge the PE)
                    lgA = h1big[64 : 64 + E, 3072 : 3072 + 256]
                    lgB = pre2[64 : 64 + E, 256:512]
                    nc.tensor.matmul(lgA, cw2[0:H, 0:E], h2[0:H, :],
                                     start=True, stop=False, tile_position=(0, 64))
                    nc.tensor.matmul(lgA, cw2[0:H, E : 2 * E], h2[0:H, :],
                                     start=False, stop=True, tile_position=(0, 64))
                    nc.tensor.matmul(lgB, cw2[H:128, 0:E], h2[H:128, :],
                                     start=True, stop=False, tile_position=(64, 64))
                    nc.tensor.matmul(lgB, cw2[H:128, E : 2 * E], h2[H:128, :],
                                     start=False, stop=True, tile_position=(64, 64))

                    if STAGE < 5:
                        continue
                    # ---- evacuate logits to zT in block-transposed column order ----
                    ztv = zT[0:E, :].rearrange("p (b r) -> p r b", r=128)
                    hw = CW // 2
                    nc.vector.tensor_copy(
                        ztv[:, CW * kg : CW * kg + hw, :],
                        lgA.rearrange("p (a b) -> p a b", a=hw),
                    )
                    nc.vector.tensor_copy(
                        ztv[:, CW * kg + hw : CW * (kg + 1), :],
                        lgB.rearrange("p (a b) -> p a b", a=hw),
                    )

            # ================= block tail =================
            if STAGE < 6:
                zer = zpool.tile([128, TW], F32, tag="zer")
                nc.vector.memset(zer[:], 0.0)
                nc.sync.dma_start(
                    out=mask_d[b0 : b0 + BLK, :].rearrange("(p w) e -> p (w e)", p=128),
                    in_=zer[:],
                )
                nc.sync.dma_start(
                    out=probs_d[b0 : b0 + BLK, :].rearrange("(p w) e -> p (w e)", p=128),
                    in_=zer[:],
                )
                continue
            # transpose zT -> natural z8 in pre2 (psum)
            for t in range(BLK // 128):
                nc.tensor.transpose(
                    pre2[:, E * t : E * (t + 1)], zT[0:E, 128 * t : 128 * (t + 1)],
                    ide[:],
                )
            gn_sb = zpool.tile([128, TW], F32, tag="gn")
            nc.sync.dma_start(
                out=gn_sb[:],
                in_=gn_d[b0 : b0 + BLK, :].rearrange("(p w) e -> p (w e)", p=128),
            )
            znat = zpool.tile([128, TW], F32, tag="znat")
            nc.vector.tensor_tensor(znat[:], pre2[:, 0:TW], gn_sb[:], op=OP.subtract)

            if STAGE < 7:
                nc.sync.dma_start(
                    out=mask_d[b0 : b0 + BLK, :].rearrange("(p w) e -> p (w e)", p=128),
                    in_=znat[:],
                )
                nc.sync.dma_start(
                    out=probs_d[b0 : b0 + BLK, :].rearrange("(p w) e -> p (w e)", p=128),
                    in_=znat[:],
                )
                continue
            zn3 = znat[:].rearrange("p (w e) -> p w e", e=E)
            mx = zpool.tile([128, W], F32, tag="mx")
            nc.vector.tensor_reduce(mx[:], zn3, axis=AX.X, op=OP.max)
            zc = zpool.tile([128, TW], F32, tag="zc")
            nc.vector.tensor_tensor(
                zc[:].rearrange("p (w e) -> p w e", e=E), zn3,
                mx[:].unsqueeze(2).broadcast_to([128, W, E]), op=OP.subtract,
            )
            ex = zpool.tile([128, TW], F32, tag="ex")
            nc.scalar.activation(ex[:], zc[:], AF.Exp, scale=1.0 / SC3)
            sm = zpool.tile([128, W], F32, tag="sm")
            nc.vector.tensor_reduce(sm[:], ex[:].rearrange("p (w e) -> p w e", e=E),
                                    axis=AX.X, op=OP.add)
            rc = zpool.tile([128, W], F32, tag="rc")
            nc.vector.reciprocal(rc[:], sm[:])
            probs = zpool.tile([128, TW], F32, tag="probs")
            nc.vector.tensor_tensor(
                probs[:].rearrange("p (w e) -> p w e", e=E),
                ex[:].rearrange("p (w e) -> p w e", e=E),
                rc[:].unsqueeze(2).broadcast_to([128, W, E]), op=OP.mult,
            )

            if STAGE < 8:
                nc.sync.dma_start(
                    out=mask_d[b0 : b0 + BLK, :].rearrange("(p w) e -> p (w e)", p=128),
                    in_=probs[:],
                )
                nc.sync.dma_start(
                    out=probs_d[b0 : b0 + BLK, :].rearrange("(p w) e -> p (w e)", p=128),
                    in_=probs[:],
                )
                continue
            # ---- rank-count top-3 ----
            pw = probs[:].rearrange("p (w e) -> p w e", e=E)
            A4 = pw.unsqueeze(2).broadcast_to([128, W, E, E])   # [.., e, j] = p_j
            B4 = pw.unsqueeze(3).broadcast_to([128, W, E, E])   # [.., e, j] = p_e
            gtm = zpool.tile([128, W * E * E], F32, tag="gtm")
            gt4 = gtm[:].rearrange("p (w e j) -> p w e j", e=E, j=E)
            nc.vector.tensor_tensor(gt4, A4, B4, op=OP.is_gt)
            eqm = zpool.tile([128, W * E * E], F32, tag="eqm")
            eq4 = eqm[:].rearrange("p (w e j) -> p w e j", e=E, j=E)
            nc.vector.tensor_tensor(eq4, A4, B4, op=OP.is_equal)
            ltb = (ltm[:].rearrange("p (e j) -> p e j", e=E)
                   .unsqueeze(1).broadcast_to([128, W, E, E]))
            eql = zpool.tile([128, W * E * E], F32, tag="eql")
            eql4 = eql[:].rearrange("p (w e j) -> p w e j", e=E, j=E)
            nc.vector.tensor_tensor(eql4, eq4, ltb, op=OP.mult)
            cnt4 = gtm  # accumulate in place: gt + eq*lt
            nc.vector.tensor_tensor(cnt4[:], gtm[:], eql[:], op=OP.add)
            cnt = zpool.tile([128, TW], F32, tag="cnt")
            nc.vector.tensor_reduce(
                cnt[:].rearrange("p (w e) -> p w e", e=E),
                cnt4[:].rearrange("p (w e j) -> p w e j", e=E, j=E),
                axis=AX.X, op=OP.add,
            )
            msk = zpool.tile([128, TW], F32, tag="msk")
            nc.vector.tensor_single_scalar(msk[:], cnt[:], float(TOPK), op=OP.is_lt)

            ms1 = zpool.tile([128, TW], F32, tag="ms1")
            nc.vector.tensor_tensor(ms1[:], msk[:], probs[:], op=OP.add)
            ste = zpool.tile([128, TW], F32, tag="ste")
            nc.vector.tensor_tensor(ste[:], ms1[:], probs[:], op=OP.subtract)

            nc.sync.dma_start(
                out=mask_d[b0 : b0 + BLK, :].rearrange("(p w) e -> p (w e)", p=128),
                in_=ste[:],
            )
            nc.sync.dma_start(
                out=probs_d[b0 : b0 + BLK, :].rearrange("(p w) e -> p (w e)", p=128),
                in_=probs[:],
            )

    nc.finalize()
    return nc


def _host_prep(contextual, u, tw1, tb1, tw2, tb2, cw1, cb1, cw2, cb2, n_examples):
    """Shared (weight) arrays + helper closures for per-core input prep."""
    f16, f32 = np.float16, np.float32
    w1 = tw1.astype(f32)
    w1h16 = w1.astype(f16)
    w1l16 = (w1 - w1h16.astype(f32)).astype(f16)
    w1hr = np.zeros((128, H), f16)
    w1lr = np.zeros((128, H), f16)
    for g in range(4):
        w1hr[32 * g : 32 * g + T] = w1h16
        w1lr[32 * g : 32 * g + T] = w1l16

    # W2[(c,h), j] = tw2[h] * cw1[c, j], scaled
    W2 = (tw2[:, 0][None, :, None] * cw1[:, None, :]).astype(f32)  # [C, H, H2=64]
    W2f = (W2.reshape(C * H, H) * SC2).astype(f32)
    W2h16 = W2f.astype(f16)
    W2l16 = (W2f - W2h16.astype(f32)).astype(f16)
    w2hS = np.zeros((128, 7 * H), f16)
    w2lS = np.zeros((128, 7 * H), f16)
    for j, (clo, chi) in enumerate(W2CHUNKS):
        w2hS[0:H, H * j : H * (j + 1)] = W2h16[clo * H : (clo + 1) * H]
        w2lS[0:H, H * j : H * (j + 1)] = W2l16[clo * H : (clo + 1) * H]
        if chi is not None:
            w2hS[H : 2 * H, H * j : H * (j + 1)] = W2h16[chi * H : (chi + 1) * H]
            w2lS[H : 2 * H, H * j : H * (j + 1)] = W2l16[chi * H : (chi + 1) * H]

    cw2f = (cw2.astype(f32) * SC3).astype(f32)
    cw2h16 = cw2f.astype(f16)
    cw2l16 = (cw2f - cw2h16.astype(f32)).astype(f16)
    cw2S = np.concatenate([cw2h16, cw2l16], axis=1)  # [64, 16]
    cw2S = np.concatenate([cw2S, cw2S], axis=0)      # [128, 16] replicated

    tb1r = np.zeros((128, 1), np.float32)
    tb1r[0:H, 0] = tb1
    tb1r[H : 2 * H, 0] = tb1
    b1p = (cb1 + tb2[0] * cw1.sum(axis=0)).astype(f32).reshape(H, 1)
    b1p = np.concatenate([b1p, b1p], axis=0)         # [128, 1] replicated

    ltmv = (np.arange(E)[None, :, None] > np.arange(E)[None, None, :]).astype(f32)
    ltm = np.broadcast_to(ltmv.reshape(1, E * E), (128, E * E)).copy()
    ide = np.eye(E, dtype=f32)

    const_map = {
        "w1h": w1hr, "w1l": w1lr, "w2h": w2hS, "w2l": w2lS, "cw2hl": cw2S,
        "tb1r": tb1r, "b1p": b1p, "ltm": ltm, "ide": ide,
    }

    X = contextual.reshape(-1, C * T)
    gn_all = (SC3 * (np.log(-np.log(u.astype(f32)) + EPS) - cb2[None, :])).astype(f32)

    def core_inputs(ci):
        s = slice(ci * n_examples, (ci + 1) * n_examples)
        Xc = X[s]
        XT = np.ascontiguousarray(Xc.T)          # [312, n] f32
        xh = XT.astype(f16)
        xl = (XT - xh.astype(f32)).astype(f16)
        return {**const_map, "xh": xh, "xl": xl, "gn8": np.ascontiguousarray(gn_all[s])}

    return core_inputs


_program_cache = {}


def _get_program(n_examples):
    if n_examples not in _program_cache:
        _program_cache[n_examples] = _build_program(n_examples)
    return _program_cache[n_examples]


def kernel(contextual, u, tw1, tb1, tw2, tb2, cw1, cb1, cw2, cb2):
    n_ex = contextual.shape[0] // N_CORES
    nc = _get_program(n_ex)
    core_inputs = _host_prep(
        np.asarray(contextual), np.asarray(u), np.asarray(tw1), np.asarray(tb1),
        np.asarray(tw2), np.asarray(tb2), np.asarray(cw1), np.asarray(cb1),
        np.asarray(cw2), np.asarray(cb2), n_ex,
    )
    in_maps = [core_inputs(ci) for ci in range(N_CORES)]
    res = run_bass_kernel_spmd(nc, in_maps, list(range(N_CORES)), trace=TRACE)
    global LAST_EXEC_NS
    LAST_EXEC_NS = res.exec_time_ns
    mask = np.concatenate([r["mask"] for r in res.results], axis=0)
    probs = np.concatenate([r["probs"] for r in res.results], axis=0)
    return mask, probs



# revision 3
# speedup vs baseline: 2.9561x; 2.9561x over previous
#!/usr/bin/env python3
"""EnvAwareRouter Trainium2 kernel (v2).

Reference computation (per example b):
  t[c]   = gelu(contextual[b,c,:] @ tw1 + tb1) @ tw2 + tb2          (C=13, T=24, H=64)
  logits = gelu(t @ cw1 + cb1) @ cw2 + cb2                          (E=8)
  probs  = softmax(logits + g),  g = -log(-log u + eps)
  mask   = k-hot(top-3 probs);  mask_ste == mask numerically

Device strategy (8 cores, pure data parallel over B=524288; all math fp16
single-term — measured 17/524288 mask flips, rel err 4.4e-3 vs 2e-2 gate):
  - host: transpose contextual to [C*T, B] fp16
  - h1: c-PAIRED matmuls, lhsT [K=48, M=128] block-diag w1 (two c's per
    streamed column), even pairs on PE row-group (0,0), odd on (64,0)
  - tw2 folded into cw1: W2[(c,h), j] = tw2[h]*cw1[c,j]; consume is a
    7-step K=832 accumulated matmul over gelu(h1); per-chunk parity
    alternates pre2 partition halves so consecutive consumes overlap
  - logits with SWAPPED operands: lhsT = h2 [K=64, M=128 examples],
    rhs = cw2 [64, 8] -> z lands NATURALLY [128 examples, 8] in PSUM
    bank 7 (timeshared with pre2); no transposes, no layout shuffles
  - top-3 via 3x max-extraction on DVE; softmax skips max-subtraction
    (z bounded); gumbel noise + cb2 folded into host-precomputed gn
  - examples are processed in a permuted order; host un-permutes outputs
"""
import sys

sys.path.insert(0, "/opt/trn_rl_repo")

import numpy as np

import concourse.bass as bass
import concourse.tile as tile
from concourse import bacc, mybir
from concourse.bass_utils import run_bass_kernel_spmd
from contextlib import ExitStack

F32 = mybir.dt.float32
F16 = mybir.dt.float16
AF = mybir.ActivationFunctionType
OP = mybir.AluOpType
AX = mybir.AxisListType

B, C, T, H, E, TOPK = 524288, 13, 24, 64, 8, 3
EPS = 1e-10
N_CORES = 8
BC = B // N_CORES          # 65536 examples per core
BLK = 8192                 # examples per block (tail granularity)
CHUNK = 512                # examples per compute chunk
DCOLS = 4096               # examples per x SBUF tile (8 chunks)
NEG = -1.0e9               # top-3 extraction knockout
TRACE = False
LAST_EXEC_NS = None


def _build_program(n_examples=BC):
    assert n_examples % BLK == 0
    nblk = n_examples // BLK           # 8
    nchunk = n_examples // CHUNK       # 128
    cpb = BLK // CHUNK                 # 16 chunks per block
    ndg = n_examples // DCOLS          # 16 x d-groups
    cpd = DCOLS // CHUNK               # 8 chunks per d-group
    TW = (BLK // 128) * E              # 512: tail tile width per block
    NE = n_examples * E // 128         # 4096: out/gn dram width
    nc = bacc.Bacc()

    xt_d = nc.declare_dram_parameter("xt", [C * T, n_examples], F16, isOutput=False)
    gn_d = nc.declare_dram_parameter("gn8", [128, NE], F32, isOutput=False)
    w1c_d = nc.declare_dram_parameter("w1c", [128, 128], F16, isOutput=False)
    w2c_d = nc.declare_dram_parameter("w2c", [128, 7 * H], F16, isOutput=False)
    cw2_d = nc.declare_dram_parameter("cw2c", [128, E], F16, isOutput=False)
    tb1_d = nc.declare_dram_parameter("tb1r", [128, 1], F32, isOutput=False)
    b1p_d = nc.declare_dram_parameter("b1pr", [128, 1], F32, isOutput=False)
    mask_d = nc.declare_dram_parameter("mask", [128, NE], F16, isOutput=True)
    probs_d = nc.declare_dram_parameter("probs", [128, NE], F16, isOutput=True)

    with tile.TileContext(nc) as tc, ExitStack() as ctx:
        cpool = ctx.enter_context(tc.tile_pool(name="consts", bufs=1))
        xpool = ctx.enter_context(tc.tile_pool(name="x", bufs=2))
        gpool = ctx.enter_context(tc.tile_pool(name="g1", bufs=2))
        hpool = ctx.enter_context(tc.tile_pool(name="h2", bufs=2))
        zpool = ctx.enter_context(tc.tile_pool(name="zblk", bufs=2))
        tpool = ctx.enter_context(tc.tile_pool(name="tail", bufs=1))
        opool = ctx.enter_context(tc.tile_pool(name="out", bufs=2))
        pspool = ctx.enter_context(tc.tile_pool(name="ps", bufs=1, space="PSUM"))

        # ---- constants ----
        w1c = cpool.tile([128, 128], F16, tag="w1c")
        nc.sync.dma_start(out=w1c[:], in_=w1c_d[:])
        w2c = cpool.tile([128, 7 * H], F16, tag="w2c")
        nc.sync.dma_start(out=w2c[:], in_=w2c_d[:])
        cw2c = cpool.tile([128, E], F16, tag="cw2c")
        nc.sync.dma_start(out=cw2c[:], in_=cw2_d[:])
        tb1r = cpool.tile([128, 1], F32, tag="tb1r")
        nc.sync.dma_start(out=tb1r[:], in_=tb1_d[:])
        b1pr = cpool.tile([128, 1], F32, tag="b1pr")
        nc.sync.dma_start(out=b1pr[:], in_=b1p_d[:])

        # ---- persistent PSUM: banks 0-6 h1 pairs, bank 7 pre2/z ----
        ps = pspool.tile([128, 4096], F32, tag="ps")
        nc.vector.memset(ps[:, 3072:3584], 0.0)   # c12 bank upper half garbage

        xts = {}
        zbs = {}
        gns = {}

        def emit_xdma(d):
            xt = xpool.tile([128, 4, DCOLS], F16, tag="xt")
            col0 = d * DCOLS
            for s in range(3):
                nc.sync.dma_start(
                    out=xt[0:48, s, :],
                    in_=xt_d[96 * s : 96 * s + 48, col0 : col0 + DCOLS],
                )
                nc.sync.dma_start(
                    out=xt[64:112, s, :],
                    in_=xt_d[96 * s + 48 : 96 * s + 96, col0 : col0 + DCOLS],
                )
            nc.sync.dma_start(
                out=xt[0:24, 3, :],
                in_=xt_d[288:312, col0 : col0 + DCOLS],
            )
            xts[d] = xt

        def emit_h1(kk):
            d, k = kk // cpd, kk % cpd
            xt = xts[d]
            off = k * CHUNK
            for s in range(3):
                nc.tensor.matmul(
                    ps[:, 1024 * s : 1024 * s + CHUNK],
                    w1c[0:48, :], xt[0:48, s, off : off + CHUNK],
                    start=True, stop=True, tile_position=(0, 0),
                )
                nc.tensor.matmul(
                    ps[:, 1024 * s + 512 : 1024 * s + 512 + CHUNK],
                    w1c[64:112, :], xt[64:112, s, off : off + CHUNK],
                    start=True, stop=True, tile_position=(64, 0),
                )
            nc.tensor.matmul(
                ps[0:64, 3072 : 3072 + CHUNK],
                w1c[0:24, 0:64], xt[0:24, 3, off : off + CHUNK],
                start=True, stop=True, tile_position=(0, 0),
            )

        def emit_gelu1(kk):
            g1 = gpool.tile([128, 3584], F16, tag="g1")
            nc.scalar.activation(g1[:, 0:2048], ps[:, 0:2048], AF.Gelu, bias=tb1r[:])
            nc.scalar.activation(g1[:, 2048:3584], ps[:, 2048:3584], AF.Gelu,
                                 bias=tb1r[:])
            return g1

        def emit_consume(kk, g1):
            pr = 64 * (kk % 2)
            for j in range(7):
                nc.tensor.matmul(
                    ps[pr : pr + 64, 3584:4096],
                    w2c[:, H * j : H * (j + 1)], g1[:, 512 * j : 512 * (j + 1)],
                    start=(j == 0), stop=(j == 6), tile_position=(0, pr),
                )

        def emit_pairtail(P):
            # gelu2 + logits + z evacuation for pair P (chunks 2P, 2P+1)
            h2 = hpool.tile([128, 512], F16, tag="h2")
            nc.scalar.activation(h2[:], ps[:, 3584:4096], AF.Gelu, bias=b1pr[:])
            for a in range(2):
                for g in range(4):
                    zi = a * 4 + g
                    nc.tensor.matmul(
                        ps[:, 3584 + 8 * zi : 3592 + 8 * zi],
                        h2[64 * a : 64 * a + 64, 128 * g : 128 * (g + 1)],
                        cw2c[64 * a : 64 * a + 64, :],
                        start=True, stop=True, tile_position=(64 * a, 0),
                    )
            b = (2 * P) // cpb
            zb = zbs[b]
            ps8 = P % 8
            nc.vector.tensor_copy(zb[:, 64 * ps8 : 64 * ps8 + 64], ps[:, 3584:3648])

        def emit_tail(b):
            zb, gnb = zbs[b], gns[b]
            znat = tpool.tile([128, TW], F32, tag="znat")
            nc.vector.tensor_tensor(znat[:], zb[:], gnb[:], op=OP.subtract)
            zn3 = znat[:].rearrange("p (w e) -> p w e", e=E)

            def bmax(src3, tag):
                m = tpool.tile([128, TW // E], F32, tag=tag)
                nc.vector.tensor_reduce(m[:], src3, axis=AX.X, op=OP.max)
                return m[:].unsqueeze(2).broadcast_to([128, TW // E, E])

            m1b = bmax(zn3, "m1")
            e1 = tpool.tile([128, TW], F32, tag="e1")
            nc.vector.tensor_tensor(e1[:].rearrange("p (w e) -> p w e", e=E), zn3,
                                    m1b, op=OP.is_ge)
            z2 = tpool.tile([128, TW], F32, tag="z2")
            nc.vector.scalar_tensor_tensor(z2[:], e1[:], NEG, znat[:],
                                           op0=OP.mult, op1=OP.add)
            z23 = z2[:].rearrange("p (w e) -> p w e", e=E)
            m2b = bmax(z23, "m2")
            e2 = tpool.tile([128, TW], F32, tag="e2")
            nc.vector.tensor_tensor(e2[:].rearrange("p (w e) -> p w e", e=E), z23,
                                    m2b, op=OP.is_ge)
            z3 = tpool.tile([128, TW], F32, tag="z3")
            nc.vector.scalar_tensor_tensor(z3[:], e2[:], NEG, z2[:],
                                           op0=OP.mult, op1=OP.add)
            m3b = bmax(z3[:].rearrange("p (w e) -> p w e", e=E), "m3")
            mask16 = opool.tile([128, TW], F16, tag="mask16")
            nc.vector.tensor_tensor(mask16[:].rearrange("p (w e) -> p w e", e=E),
                                    zn3, m3b, op=OP.is_ge)
            nc.sync.dma_start(out=mask_d[:, TW * b : TW * (b + 1)], in_=mask16[:])

            pex = tpool.tile([128, TW], F32, tag="pex")
            nc.scalar.activation(pex[:], znat[:], AF.Exp)
            sm = tpool.tile([128, TW // E], F32, tag="sm")
            nc.vector.tensor_reduce(sm[:], pex[:].rearrange("p (w e) -> p w e", e=E),
                                    axis=AX.X, op=OP.add)
            rc = tpool.tile([128, TW // E], F32, tag="rc")
            nc.vector.reciprocal(rc[:], sm[:])
            probs16 = opool.tile([128, TW], F16, tag="probs16")
            nc.vector.tensor_tensor(
                probs16[:].rearrange("p (w e) -> p w e", e=E),
                pex[:].rearrange("p (w e) -> p w e", e=E),
                rc[:].unsqueeze(2).broadcast_to([128, TW // E, E]), op=OP.mult,
            )
            nc.sync.dma_start(out=probs_d[:, TW * b : TW * (b + 1)], in_=probs16[:])

        # ---- main pipeline ----
        emit_xdma(0)
        emit_h1(0)
        g1 = None
        for kk in range(nchunk):
            if kk % cpd == 0 and kk // cpd + 1 < ndg:
                emit_xdma(kk // cpd + 1)
            if kk % cpb == 0:
                b = kk // cpb
                gnb = zpool.tile([128, TW], F32, tag="gnb")
                nc.sync.dma_start(out=gnb[:], in_=gn_d[:, TW * b : TW * (b + 1)])
                gns[b] = gnb
                zbs[b] = zpool.tile([128, TW], F32, tag="zb", name="zb")
            g1 = emit_gelu1(kk)
            if kk + 1 < nchunk:
                emit_h1(kk + 1)
            if kk >= 2 and kk % 2 == 0:
                emit_pairtail(kk // 2 - 1)
            emit_consume(kk, g1)
            if kk % cpb == 2 and kk >= cpb:
                emit_tail(kk // cpb - 1)
        emit_pairtail(nchunk // 2 - 1)
        emit_tail(nblk - 1)

    nc.finalize()
    return nc


def _host_prep(contextual, u, tw1, tb1, tw2, tb2, cw1, cb1, cw2, cb2, n_examples):
    f16, f32 = np.float16, np.float32

    # w1c: block-diag pair weights, replicated at partition 0 and 64
    w1blk = np.zeros((48, 128), f16)
    w1blk[0:24, 0:64] = tw1.astype(f16)
    w1blk[24:48, 64:128] = tw1.astype(f16)
    w1c = np.zeros((128, 128), f16)
    w1c[0:48] = w1blk
    w1c[64:112] = w1blk

    # w2c[p, 64j+m]: p<64 -> c=2j,h=p ; p>=64 -> c=2j+1,h=p-64 (j=6 upper: 0)
    W2 = (tw2[:, 0][None, :, None] * cw1[:, None, :]).astype(f32)  # [C, H, 64]
    w2c = np.zeros((128, 7 * H), f16)
    for j in range(7):
        clo = 2 * j
        w2c[0:64, H * j : H * (j + 1)] = W2[clo].astype(f16)
        if clo + 1 < C:
            w2c[64:128, H * j : H * (j + 1)] = W2[clo + 1].astype(f16)

    cw2c = np.concatenate([cw2.astype(f16), cw2.astype(f16)], axis=0)  # [128, 8]

    tb1r = np.tile(tb1.astype(f32), 2).reshape(128, 1)
    b1p = (cb1 + tb2[0] * cw1.sum(axis=0)).astype(f32)
    b1pr = np.tile(b1p, 2).reshape(128, 1)

    const_map = {
        "w1c": w1c, "w2c": w2c, "cw2c": cw2c, "tb1r": tb1r, "b1pr": b1pr,
    }

    X = contextual.reshape(-1, C * T)
    # gn = -(g + cb2) = log(-log u + eps) - cb2 ; device computes z - gn
    gn_all = (np.log(-np.log(u.astype(f32)) + EPS) - cb2[None, :]).astype(f32)

    nch = n_examples // CHUNK

    def core_inputs(ci):
        s = slice(ci * n_examples, (ci + 1) * n_examples)
        xt = np.ascontiguousarray(X[s].T).astype(f16)     # [312, n]
        gn = gn_all[s]                                    # [n, 8]
        # device order: ex = ch*512 + g*128 + p -> gn_dev[p, (ch*4+g)*8+e]
        gn_dev = np.ascontiguousarray(
            gn.reshape(nch, 4, 128, E).transpose(2, 0, 1, 3).reshape(128, -1)
        )
        return {**const_map, "xt": xt, "gn8": gn_dev}

    return core_inputs


_program_cache = {}


def _get_program(n_examples):
    if n_examples not in _program_cache:
        _program_cache[n_examples] = _build_program(n_examples)
    return _program_cache[n_examples]


def kernel(contextual, u, tw1, tb1, tw2, tb2, cw1, cb1, cw2, cb2):
    n_ex = contextual.shape[0] // N_CORES
    nc = _get_program(n_ex)
    core_inputs = _host_prep(
        np.asarray(contextual), np.asarray(u), np.asarray(tw1), np.asarray(tb1),
        np.asarray(tw2), np.asarray(tb2), np.asarray(cw1), np.asarray(cb1),
        np.asarray(cw2), np.asarray(cb2), n_ex,
    )
    in_maps = [core_inputs(ci) for ci in range(N_CORES)]
    res = run_bass_kernel_spmd(nc, in_maps, list(range(N_CORES)), trace=TRACE)
    global LAST_EXEC_NS
    LAST_EXEC_NS = res.exec_time_ns
    nch = n_ex // CHUNK
    outs = []
    for key in ("mask", "probs"):
        full = np.empty((N_CORES * n_ex, E), np.float32)
        for ci in range(N_CORES):
            dev = res.results[ci][key].astype(np.float32)   # [128, n*8/128]
            # invert: dev[p, (ch*4+g)*8+e] -> ex = ch*512+g*128+p
            full[ci * n_ex : (ci + 1) * n_ex] = (
                dev.reshape(128, nch, 4, E).transpose(1, 2, 0, 3).reshape(n_ex, E)
            )
        outs.append(full)
    return outs[0], outs[1]


# revision 7
# speedup vs baseline: 2.9725x; 1.0055x over previous
#!/usr/bin/env python3
"""EnvAwareRouter Trainium2 kernel (v2).

Reference computation (per example b):
  t[c]   = gelu(contextual[b,c,:] @ tw1 + tb1) @ tw2 + tb2          (C=13, T=24, H=64)
  logits = gelu(t @ cw1 + cb1) @ cw2 + cb2                          (E=8)
  probs  = softmax(logits + g),  g = -log(-log u + eps)
  mask   = k-hot(top-3 probs);  mask_ste == mask numerically

Device strategy (8 cores, pure data parallel over B=524288; all math fp16
single-term — measured 17/524288 mask flips, rel err 4.4e-3 vs 2e-2 gate):
  - host: transpose contextual to [C*T, B] fp16
  - h1: c-PAIRED matmuls, lhsT [K=48, M=128] block-diag w1 (two c's per
    streamed column), even pairs on PE row-group (0,0), odd on (64,0)
  - tw2 folded into cw1: W2[(c,h), j] = tw2[h]*cw1[c,j]; consume is a
    7-step K=832 accumulated matmul over gelu(h1); per-chunk parity
    alternates pre2 partition halves so consecutive consumes overlap
  - logits with SWAPPED operands: lhsT = h2 [K=64, M=128 examples],
    rhs = cw2 [64, 8] -> z lands NATURALLY [128 examples, 8] in PSUM
    bank 7 (timeshared with pre2); no transposes, no layout shuffles
  - top-3 via 3x max-extraction on DVE; softmax skips max-subtraction
    (z bounded); gumbel noise + cb2 folded into host-precomputed gn
  - examples are processed in a permuted order; host un-permutes outputs
"""
import sys

sys.path.insert(0, "/opt/trn_rl_repo")

import numpy as np

import concourse.bass as bass
import concourse.tile as tile
from concourse import bacc, mybir
from concourse.bass_utils import run_bass_kernel_spmd
from contextlib import ExitStack

F32 = mybir.dt.float32
F16 = mybir.dt.float16
AF = mybir.ActivationFunctionType
OP = mybir.AluOpType
AX = mybir.AxisListType

B, C, T, H, E, TOPK = 524288, 13, 24, 64, 8, 3
EPS = 1e-10
N_CORES = 8
BC = B // N_CORES          # 65536 examples per core
BLK = 8192                 # examples per block (tail granularity)
CHUNK = 512                # examples per compute chunk
DCOLS = 4096               # examples per x SBUF tile (8 chunks)
NEG = -1.0e9               # top-3 extraction knockout
TRACE = False
LAST_EXEC_NS = None


def _build_program(n_examples=BC):
    assert n_examples % BLK == 0
    nblk = n_examples // BLK           # 8
    nchunk = n_examples // CHUNK       # 128
    cpb = BLK // CHUNK                 # 16 chunks per block
    ndg = n_examples // DCOLS          # 16 x d-groups
    cpd = DCOLS // CHUNK               # 8 chunks per d-group
    TW = (BLK // 128) * E              # 512: tail tile width per block
    NE = n_examples * E // 128         # 4096: out/gn dram width
    nc = bacc.Bacc()

    xt_d = nc.declare_dram_parameter("xt", [C * T, n_examples], F16, isOutput=False)
    gn_d = nc.declare_dram_parameter("gn8", [128, NE], F32, isOutput=False)
    w1c_d = nc.declare_dram_parameter("w1c", [128, 128], F16, isOutput=False)
    w2c_d = nc.declare_dram_parameter("w2c", [128, 7 * H], F16, isOutput=False)
    cw2_d = nc.declare_dram_parameter("cw2c", [128, E], F16, isOutput=False)
    tb1_d = nc.declare_dram_parameter("tb1r", [128, 1], F32, isOutput=False)
    b1p_d = nc.declare_dram_parameter("b1pr", [128, 1], F32, isOutput=False)
    mask_d = nc.declare_dram_parameter("mask", [128, NE], F16, isOutput=True)
    probs_d = nc.declare_dram_parameter("probs", [128, NE], F16, isOutput=True)

    with tile.TileContext(nc) as tc, ExitStack() as ctx:
        cpool = ctx.enter_context(tc.tile_pool(name="consts", bufs=1))
        xpool = ctx.enter_context(tc.tile_pool(name="x", bufs=2))
        gpool = ctx.enter_context(tc.tile_pool(name="g1", bufs=2))
        hpool = ctx.enter_context(tc.tile_pool(name="h2", bufs=2))
        zpool = ctx.enter_context(tc.tile_pool(name="zblk", bufs=4))
        tpool = ctx.enter_context(tc.tile_pool(name="tail", bufs=2))
        opool = ctx.enter_context(tc.tile_pool(name="out", bufs=2))
        pspool = ctx.enter_context(tc.tile_pool(name="ps", bufs=1, space="PSUM"))

        # ---- constants ----
        w1c = cpool.tile([128, 128], F16, tag="w1c")
        nc.sync.dma_start(out=w1c[:], in_=w1c_d[:])
        w2c = cpool.tile([128, 7 * H], F16, tag="w2c")
        nc.sync.dma_start(out=w2c[:], in_=w2c_d[:])
        cw2c = cpool.tile([128, E], F16, tag="cw2c")
        nc.sync.dma_start(out=cw2c[:], in_=cw2_d[:])
        tb1r = cpool.tile([128, 1], F32, tag="tb1r")
        nc.sync.dma_start(out=tb1r[:], in_=tb1_d[:])
        b1pr = cpool.tile([128, 1], F32, tag="b1pr")
        nc.sync.dma_start(out=b1pr[:], in_=b1p_d[:])

        # ---- persistent PSUM: banks 0-6 h1 pairs, bank 7 pre2/z ----
        ps = pspool.tile([128, 4096], F32, tag="ps")
        nc.vector.memset(ps[:, 3072:3584], 0.0)   # c12 bank upper half garbage

        xts = {}
        zbs = {}
        gns = {}

        def emit_xdma(d):
            xt = xpool.tile([128, 4, DCOLS], F16, tag="xt")
            col0 = d * DCOLS
            # first d-group: chunk-granular pieces so h1(0) starts early
            pieces = (
                [(c * CHUNK, (c + 1) * CHUNK) for c in range(cpd)]
                if d == 0 else [(0, DCOLS)]
            )
            for lo, hi in pieces:
                for s in range(3):
                    nc.sync.dma_start(
                        out=xt[0:48, s, lo:hi],
                        in_=xt_d[96 * s : 96 * s + 48, col0 + lo : col0 + hi],
                    )
                    nc.sync.dma_start(
                        out=xt[64:112, s, lo:hi],
                        in_=xt_d[96 * s + 48 : 96 * s + 96, col0 + lo : col0 + hi],
                    )
                nc.sync.dma_start(
                    out=xt[0:24, 3, lo:hi],
                    in_=xt_d[288:312, col0 + lo : col0 + hi],
                )
            xts[d] = xt

        def emit_h1(kk):
            d, k = kk // cpd, kk % cpd
            xt = xts[d]
            off = k * CHUNK
            for s in range(3):
                nc.tensor.matmul(
                    ps[:, 1024 * s : 1024 * s + CHUNK],
                    w1c[0:48, :], xt[0:48, s, off : off + CHUNK],
                    start=True, stop=True, tile_position=(0, 0),
                )
                nc.tensor.matmul(
                    ps[:, 1024 * s + 512 : 1024 * s + 512 + CHUNK],
                    w1c[64:112, :], xt[64:112, s, off : off + CHUNK],
                    start=True, stop=True, tile_position=(64, 0),
                )
            nc.tensor.matmul(
                ps[0:64, 3072 : 3072 + CHUNK],
                w1c[0:24, 0:64], xt[0:24, 3, off : off + CHUNK],
                start=True, stop=True, tile_position=(0, 0),
            )

        def emit_gelu1(kk):
            g1 = gpool.tile([128, 3584], F16, tag="g1")
            nc.scalar.activation(g1[:, 0:2048], ps[:, 0:2048], AF.Gelu, bias=tb1r[:])
            nc.scalar.activation(g1[:, 2048:3584], ps[:, 2048:3584], AF.Gelu,
                                 bias=tb1r[:])
            return g1

        def emit_consume(kk, g1):
            pr = 64 * (kk % 2)
            for j in range(7):
                nc.tensor.matmul(
                    ps[pr : pr + 64, 3584:4096],
                    w2c[:, H * j : H * (j + 1)], g1[:, 512 * j : 512 * (j + 1)],
                    start=(j == 0), stop=(j == 6), tile_position=(0, pr),
                )

        def emit_pairtail(P):
            # gelu2 + logits + z evacuation for pair P (chunks 2P, 2P+1)
            h2 = hpool.tile([128, 512], F16, tag="h2")
            nc.scalar.activation(h2[:], ps[:, 3584:4096], AF.Gelu, bias=b1pr[:])
            for a in range(2):
                for g in range(4):
                    zi = a * 4 + g
                    nc.tensor.matmul(
                        ps[:, 3584 + 8 * zi : 3592 + 8 * zi],
                        h2[64 * a : 64 * a + 64, 128 * g : 128 * (g + 1)],
                        cw2c[64 * a : 64 * a + 64, :],
                        start=True, stop=True, tile_position=(64 * a, 0),
                    )
            b = (2 * P) // cpb
            zb = zbs[b]
            ps8 = P % 8
            nc.vector.tensor_copy(zb[:, 64 * ps8 : 64 * ps8 + 64], ps[:, 3584:3648])

        def emit_tail(b):
            zb, gnb = zbs[b], gns[b]
            znat = tpool.tile([128, TW], F32, tag="znat")
            nc.vector.tensor_tensor(znat[:], zb[:], gnb[:], op=OP.subtract)
            zn3 = znat[:].rearrange("p (w e) -> p w e", e=E)

            def bmax(src3, tag):
                m = tpool.tile([128, TW // E], F32, tag=tag)
                nc.vector.tensor_reduce(m[:], src3, axis=AX.X, op=OP.max)
                return m[:].unsqueeze(2).broadcast_to([128, TW // E, E])

            m1b = bmax(zn3, "m1")
            e1 = tpool.tile([128, TW], F32, tag="e1")
            nc.vector.tensor_tensor(e1[:].rearrange("p (w e) -> p w e", e=E), zn3,
                                    m1b, op=OP.is_ge)
            z2 = tpool.tile([128, TW], F32, tag="z2")
            nc.vector.scalar_tensor_tensor(z2[:], e1[:], NEG, znat[:],
                                           op0=OP.mult, op1=OP.add)
            z23 = z2[:].rearrange("p (w e) -> p w e", e=E)
            m2b = bmax(z23, "m2")
            e2 = tpool.tile([128, TW], F32, tag="e2")
            nc.vector.tensor_tensor(e2[:].rearrange("p (w e) -> p w e", e=E), z23,
                                    m2b, op=OP.is_ge)
            z3 = tpool.tile([128, TW], F32, tag="z3")
            nc.vector.scalar_tensor_tensor(z3[:], e2[:], NEG, z2[:],
                                           op0=OP.mult, op1=OP.add)
            m3b = bmax(z3[:].rearrange("p (w e) -> p w e", e=E), "m3")
            mask16 = opool.tile([128, TW], F16, tag="mask16")
            nc.vector.tensor_tensor(mask16[:].rearrange("p (w e) -> p w e", e=E),
                                    zn3, m3b, op=OP.is_ge)
            nc.sync.dma_start(out=mask_d[:, TW * b : TW * (b + 1)], in_=mask16[:])

            pex = tpool.tile([128, TW], F32, tag="pex")
            nc.scalar.activation(pex[:], znat[:], AF.Exp)
            sm = tpool.tile([128, TW // E], F32, tag="sm")
            nc.vector.tensor_reduce(sm[:], pex[:].rearrange("p (w e) -> p w e", e=E),
                                    axis=AX.X, op=OP.add)
            rc = tpool.tile([128, TW // E], F32, tag="rc")
            nc.vector.reciprocal(rc[:], sm[:])
            probs16 = opool.tile([128, TW], F16, tag="probs16")
            nc.vector.tensor_tensor(
                probs16[:].rearrange("p (w e) -> p w e", e=E),
                pex[:].rearrange("p (w e) -> p w e", e=E),
                rc[:].unsqueeze(2).broadcast_to([128, TW // E, E]), op=OP.mult,
            )
            nc.sync.dma_start(out=probs_d[:, TW * b : TW * (b + 1)], in_=probs16[:])

        # ---- main pipeline ----
        emit_xdma(0)
        emit_h1(0)
        g1 = None
        for kk in range(nchunk):
            if kk % cpd == 0 and kk // cpd + 1 < ndg:
                emit_xdma(kk // cpd + 1)
            if kk % cpb == 0:
                b = kk // cpb
                gnb = zpool.tile([128, TW], F32, tag="gnb")
                nc.sync.dma_start(out=gnb[:], in_=gn_d[:, TW * b : TW * (b + 1)])
                gns[b] = gnb
                zbs[b] = zpool.tile([128, TW], F32, tag="zb", name="zb")
            g1 = emit_gelu1(kk)
            if kk + 1 < nchunk:
                emit_h1(kk + 1)
            if kk >= 2 and kk % 2 == 0:
                emit_pairtail(kk // 2 - 1)
            emit_consume(kk, g1)
            # tails batched per 2 blocks: one exp/gelu table round-trip each
            if kk % (2 * cpb) == 2 and kk >= 2 * cpb:
                sb = kk // (2 * cpb) - 1
                emit_tail(2 * sb)
                emit_tail(2 * sb + 1)
        emit_pairtail(nchunk // 2 - 1)
        emit_tail(nblk - 2)
        emit_tail(nblk - 1)

    nc.finalize()
    return nc


def _host_prep(contextual, u, tw1, tb1, tw2, tb2, cw1, cb1, cw2, cb2, n_examples):
    f16, f32 = np.float16, np.float32

    # w1c: block-diag pair weights, replicated at partition 0 and 64
    w1blk = np.zeros((48, 128), f16)
    w1blk[0:24, 0:64] = tw1.astype(f16)
    w1blk[24:48, 64:128] = tw1.astype(f16)
    w1c = np.zeros((128, 128), f16)
    w1c[0:48] = w1blk
    w1c[64:112] = w1blk

    # w2c[p, 64j+m]: p<64 -> c=2j,h=p ; p>=64 -> c=2j+1,h=p-64 (j=6 upper: 0)
    W2 = (tw2[:, 0][None, :, None] * cw1[:, None, :]).astype(f32)  # [C, H, 64]
    w2c = np.zeros((128, 7 * H), f16)
    for j in range(7):
        clo = 2 * j
        w2c[0:64, H * j : H * (j + 1)] = W2[clo].astype(f16)
        if clo + 1 < C:
            w2c[64:128, H * j : H * (j + 1)] = W2[clo + 1].astype(f16)

    cw2c = np.concatenate([cw2.astype(f16), cw2.astype(f16)], axis=0)  # [128, 8]

    tb1r = np.tile(tb1.astype(f32), 2).reshape(128, 1)
    b1p = (cb1 + tb2[0] * cw1.sum(axis=0)).astype(f32)
    b1pr = np.tile(b1p, 2).reshape(128, 1)

    const_map = {
        "w1c": w1c, "w2c": w2c, "cw2c": cw2c, "tb1r": tb1r, "b1pr": b1pr,
    }

    X = contextual.reshape(-1, C * T)
    # gn = -(g + cb2) = log(-log u + eps) - cb2 ; device computes z - gn
    gn_all = (np.log(-np.log(u.astype(f32)) + EPS) - cb2[None, :]).astype(f32)

    nch = n_examples // CHUNK

    def core_inputs(ci):
        s = slice(ci * n_examples, (ci + 1) * n_examples)
        xt = np.ascontiguousarray(X[s].T).astype(f16)     # [312, n]
        gn = gn_all[s]                                    # [n, 8]
        # device order: ex = ch*512 + g*128 + p -> gn_dev[p, (ch*4+g)*8+e]
        gn_dev = np.ascontiguousarray(
            gn.reshape(nch, 4, 128, E).transpose(2, 0, 1, 3).reshape(128, -1)
        )
        return {**const_map, "xt": xt, "gn8": gn_dev}

    return core_inputs


_program_cache = {}


def _get_program(n_examples):
    if n_examples not in _program_cache:
        _program_cache[n_examples] = _build_program(n_examples)
    return _program_cache[n_examples]


def kernel(contextual, u, tw1, tb1, tw2, tb2, cw1, cb1, cw2, cb2):
    n_ex = contextual.shape[0] // N_CORES
    nc = _get_program(n_ex)
    core_inputs = _host_prep(
        np.asarray(contextual), np.asarray(u), np.asarray(tw1), np.asarray(tb1),
        np.asarray(tw2), np.asarray(tb2), np.asarray(cw1), np.asarray(cb1),
        np.asarray(cw2), np.asarray(cb2), n_ex,
    )
    in_maps = [core_inputs(ci) for ci in range(N_CORES)]
    res = run_bass_kernel_spmd(nc, in_maps, list(range(N_CORES)), trace=TRACE)
    global LAST_EXEC_NS
    LAST_EXEC_NS = res.exec_time_ns
    nch = n_ex // CHUNK
    outs = []
    for key in ("mask", "probs"):
        full = np.empty((N_CORES * n_ex, E), np.float32)
        for ci in range(N_CORES):
            dev = res.results[ci][key].astype(np.float32)   # [128, n*8/128]
            # invert: dev[p, (ch*4+g)*8+e] -> ex = ch*512+g*128+p
            full[ci * n_ex : (ci + 1) * n_ex] = (
                dev.reshape(128, nch, 4, E).transpose(1, 2, 0, 3).reshape(n_ex, E)
            )
        outs.append(full)
    return outs[0], outs[1]
